# revision 5
# baseline (speedup 1.0000x reference)
"""Trainium2 Bass kernel for nn_Cal_Div_Loss (conv-pyramid L1 loss).

Strategy
--------
The 3x3 all-ones stride-2 VALID conv ("edgesum") is linear, so the x- and
y-pyramids collapse into a single pyramid over d = x - y.  Per sample we
need sum(d) (for the 'last' column) and sum(|d_l|) at 5 pyramid levels
(512 -> 255 -> 127 -> 63 -> 31).  The tiny cross-batch 'fuhao' sign logic
and the final mean are O(B*6) and run on the host.

Sharding: data-parallel over batch, 64 samples / 8 cores = 8 samples/core.
Per core 16 MiB of input -> DMA-bound at ~358 GB/s (~47 us) — the target.

Per level, edgesum(d) = R @ d @ R^T (R = banded ones, window 3 stride 2):
  - column-window sum (d @ R^T) = two strided tensor_tensor adds on DVE,
    SBUF -> SBUF
  - row-window sum (R @ .) = matmuls with the banded R^T chunks as the
    stationary operand, accumulated over 128-row chunks in PSUM
  - ACT evacuates each PSUM level image into batched SBUF tiles
    [P, 8, N] and also does |d| accumulation at level 0
  - deep-level column sums and |.| stats run batched over all 8 samples
  - a few samples' subtract runs on GPSIMD (fused with the signed sum via
    scalar_tensor_tensor accum_out) to unload DVE
"""

import sys

if "/opt/trn_rl_repo" not in sys.path:
    sys.path.insert(0, "/opt/trn_rl_repo")

import numpy as np

# ---------------------------------------------------------------- constants
B = 64          # full batch
NCORES = 8
S = B // NCORES  # samples per core
P = 128
N0, N1, N2, N3, N4 = 512, 255, 127, 63, 31
G0 = 4          # 128-row chunks at level 0
LAYER_NUM = 4

# samples whose subtract runs on GPSIMD instead of DVE (load balancing)
GPSIMD_SUB_SAMPLES = (5, 6, 7)

# stats tile columns: [0:8] sd, [8:16] sa0, [16:24] sa1 rows 0..127,
# [24:32] sa1 rows 128..254, [32:40] sa2, [40:48] sa3, [48:56] sa4
STATS_COLS = 64

_CACHE = {}


def _banded(n_out, n_in, pad_to=None):
    """R^T for the window-3 stride-2 row sum: [n_in, n_out] fp32."""
    r = np.zeros((n_out, n_in), dtype=np.float32)
    for i in range(n_out):
        r[i, 2 * i : 2 * i + 3] = 1.0
    bt = np.ascontiguousarray(r.T)
    if pad_to is not None and pad_to > n_in:
        bt = np.concatenate(
            [bt, np.zeros((pad_to - n_in, n_out), dtype=np.float32)], axis=0
        )
    return bt


def _colsum(nc, out, src):
    """out = src[..., 0::2] + src[..., 1::2] + src[..., 2::2] (win 3 stride 2).

    src must be in SBUF (both DVE read ports hit the same tile)."""
    n_in = src.shape[-1]
    n_out = out.shape[-1]
    assert n_out == (n_in - 3) // 2 + 1
    sl = [slice(None)] * (len(src.shape) - 1)
    e0 = src[tuple(sl + [slice(0, 2 * n_out - 1, 2)])]
    e1 = src[tuple(sl + [slice(1, 2 * n_out, 2)])]
    e2 = src[tuple(sl + [slice(2, 2 * n_out + 1, 2)])]
    nc.vector.tensor_add(out=out, in0=e0, in1=e1)
    nc.vector.tensor_add(out=out, in0=out, in1=e2)


def _build_nc():
    from contextlib import ExitStack

    import concourse.bacc as bacc
    import concourse.mybir as mybir
    import concourse.tile as tile

    f32 = mybir.dt.float32
    SUB = mybir.AluOpType.subtract
    ADD = mybir.AluOpType.add
    AX = mybir.AxisListType.X
    AF = mybir.ActivationFunctionType

    nc = bacc.Bacc("TRN2", target_bir_lowering=False, debug=False)
    xs = nc.dram_tensor("xs", [S, 512, 512], f32, kind="ExternalInput").ap()
    ys = nc.dram_tensor("ys", [S, 512, 512], f32, kind="ExternalInput").ap()
    bt0 = nc.dram_tensor("bt0", [512, N1], f32, kind="ExternalInput").ap()
    bt1 = nc.dram_tensor("bt1", [256, N2], f32, kind="ExternalInput").ap()
    bt2 = nc.dram_tensor("bt2", [N2, N3], f32, kind="ExternalInput").ap()
    bt3 = nc.dram_tensor("bt3", [N3, N4], f32, kind="ExternalInput").ap()
    stats_out = nc.dram_tensor(
        "stats", [P, STATS_COLS], f32, kind="ExternalOutput"
    ).ap()

    with tile.TileContext(nc) as tc, ExitStack() as ctx:
        singles = ctx.enter_context(tc.tile_pool(name="singles", bufs=1))
        xy = ctx.enter_context(tc.tile_pool(name="xy", bufs=3))
        dpool = ctx.enter_context(tc.tile_pool(name="d", bufs=2))
        scr = ctx.enter_context(tc.tile_pool(name="scr", bufs=1))
        pd1 = ctx.enter_context(tc.tile_pool(name="pd1", bufs=4, space="PSUM"))
        pd2 = ctx.enter_context(tc.tile_pool(name="pd2", bufs=2, space="PSUM"))
        pd34 = ctx.enter_context(tc.tile_pool(name="pd34", bufs=2, space="PSUM"))

        # banded-ones constants (stationary matmul operands)
        bt0_sb = singles.tile([P, G0, N1], f32)
        nc.sync.dma_start(out=bt0_sb, in_=bt0.rearrange("(g p) i -> p g i", p=P))
        bt1_sb = singles.tile([P, 2, N2], f32)
        nc.sync.dma_start(out=bt1_sb, in_=bt1.rearrange("(g p) i -> p g i", p=P))
        bt2_sb = singles.tile([N2, N3], f32)
        nc.sync.dma_start(out=bt2_sb, in_=bt2)
        bt3_sb = singles.tile([N3, N4], f32)
        nc.sync.dma_start(out=bt3_sb, in_=bt3)

        # persistent per-level images / col-sums, batched over samples
        d1a = singles.tile([P, S, N1], f32)    # d1 rows 0..127
        d1b = singles.tile([127, S, N1], f32)  # d1 rows 128..254
        v1a = singles.tile([P, S, N2], f32)
        v1b = singles.tile([127, S, N2], f32)
        d2A = singles.tile([N2, S, N2], f32)
        v2A = singles.tile([N2, S, N3], f32)
        d3A = singles.tile([N3, S, N3], f32)
        v3A = singles.tile([N3, S, N4], f32)
        d4A = singles.tile([N4, S, N4], f32)
        stats = singles.tile([P, STATS_COLS], f32)
        nc.vector.memset(stats, 0.0)

        # ---------------- phase 0: per-sample level-0 work ----------------
        for s in range(S):
            xt = xy.tile([P, G0, N0], f32, tag="xt")
            yt = xy.tile([P, G0, N0], f32, tag="yt")
            nc.sync.dma_start(out=xt, in_=xs[s].rearrange("(g p) c -> p g c", p=P))
            nc.sync.dma_start(out=yt, in_=ys[s].rearrange("(g p) c -> p g c", p=P))

            dt = dpool.tile([P, G0, N0], f32, tag="dt")
            if s in GPSIMD_SUB_SAMPLES:
                nc.gpsimd.tensor_sub(out=dt, in0=xt, in1=yt)
                cscr = scr.tile([P, G0, N0], f32, tag="cscr")
                nc.scalar.activation(
                    out=cscr, in_=dt, func=AF.Copy,
                    accum_out=stats[:, s : s + 1],
                )
            else:
                nc.vector.scalar_tensor_tensor(
                    out=dt, in0=xt, scalar=0.0, in1=yt,
                    op0=ADD, op1=SUB, accum_out=stats[:, s : s + 1],
                )

            ascr = scr.tile([P, G0, N0], f32, tag="ascr")
            nc.scalar.activation(
                out=ascr, in_=dt, func=AF.Abs,
                accum_out=stats[:, 8 + s : 9 + s],
            )

            # col-window sum (SBUF->SBUF): v0 [P, G0, N1]
            v0 = dpool.tile([P, G0, N1], f32, tag="v0")
            _colsum(nc, v0, dt)

            # row-window sums on PE: d1 = R0 @ v0, two 128-row out chunks
            for m, gs in ((0, (0, 1, 2)), (1, (2, 3))):
                mp = 128 if m == 0 else 127
                w = pd1.tile([P, N1], f32, tag="pd1")
                for j, g in enumerate(gs):
                    nc.tensor.matmul(
                        w[:mp, :],
                        bt0_sb[:, g, m * 128 : m * 128 + mp],
                        v0[:, g, :],
                        start=(j == 0),
                        stop=(j == len(gs) - 1),
                    )
                # evacuate PSUM -> batched SBUF tile
                tgt = (d1a if m == 0 else d1b)[:, s, :]
                nc.scalar.copy(out=tgt, in_=w[:mp, :])

        # ---------------- phase 1: level 1 (batched colsum + PE) ----------
        _colsum(nc, v1a, d1a)
        _colsum(nc, v1b, d1b)
        for s in range(S):
            w = pd2.tile([N2, N2], f32, tag="pd2")
            nc.tensor.matmul(w, bt1_sb[:, 0, :], v1a[:, s, :], start=True, stop=False)
            nc.tensor.matmul(
                w, bt1_sb[0:127, 1, :], v1b[:, s, :], start=False, stop=True
            )
            nc.scalar.copy(out=d2A[:, s, :], in_=w)

        # ---------------- phase 2: level 2 --------------------------------
        _colsum(nc, v2A, d2A)
        for s in range(S):
            w = pd34.tile([N3, N3], f32, tag="pd34")
            nc.tensor.matmul(w, bt2_sb, v2A[:, s, :], start=True, stop=True)
            nc.scalar.copy(out=d3A[:, s, :], in_=w)

        # ---------------- phase 3: level 3 --------------------------------
        _colsum(nc, v3A, d3A)
        for s in range(S):
            w = pd34.tile([N4, N4], f32, tag="pd34")
            nc.tensor.matmul(w, bt3_sb, v3A[:, s, :], start=True, stop=True)
            nc.scalar.copy(out=d4A[:, s, :], in_=w)

        # ---------------- batched |.| stats for levels 1..4 ---------------
        nc.vector.tensor_reduce(
            out=stats[:, 16:24], in_=d1a, axis=AX, op=ADD,
            apply_absolute_value=True,
        )
        nc.vector.tensor_reduce(
            out=stats[0:127, 24:32], in_=d1b, axis=AX, op=ADD,
            apply_absolute_value=True,
        )
        nc.vector.tensor_reduce(
            out=stats[0:127, 32:40], in_=d2A, axis=AX, op=ADD,
            apply_absolute_value=True,
        )
        nc.vector.tensor_reduce(
            out=stats[0:63, 40:48], in_=d3A, axis=AX, op=ADD,
            apply_absolute_value=True,
        )
        nc.vector.tensor_reduce(
            out=stats[0:31, 48:56], in_=d4A, axis=AX, op=ADD,
            apply_absolute_value=True,
        )

        nc.sync.dma_start(out=stats_out, in_=stats)

    nc.finalize()
    return nc


def _get_nc():
    if "nc" not in _CACHE:
        _CACHE["nc"] = _build_nc()
    return _CACHE["nc"]


def _run_on_hw(x, y, trace=False):
    """x, y: [64, 512, 512] fp32 numpy. Returns list of 8 stats arrays."""
    from concourse.bass_utils import run_bass_kernel_spmd

    nc = _get_nc()
    bt0 = _banded(N1, 512)
    bt1 = _banded(N2, N1, pad_to=256)
    bt2 = _banded(N3, N2)
    bt3 = _banded(N4, N3)

    in_maps = []
    for c in range(NCORES):
        in_maps.append(
            {
                "xs": np.ascontiguousarray(x[c * S : (c + 1) * S]),
                "ys": np.ascontiguousarray(y[c * S : (c + 1) * S]),
                "bt0": bt0,
                "bt1": bt1,
                "bt2": bt2,
                "bt3": bt3,
            }
        )

    res = run_bass_kernel_spmd(
        nc, in_maps, core_ids=list(range(NCORES)), trace=trace
    )
    _CACHE["last_results"] = res
    return [r["stats"] for r in res.results]


def kernel(x, y, alpha, _trace=False):
    x = np.ascontiguousarray(np.asarray(x, dtype=np.float32).reshape(B, 512, 512))
    y = np.ascontiguousarray(np.asarray(y, dtype=np.float32).reshape(B, 512, 512))
    alpha = np.asarray(alpha, dtype=np.float32)

    stats_list = _run_on_hw(x, y, trace=_trace)

    sd = np.empty(B, np.float64)
    sa = np.empty((B, 5), np.float64)
    for c in range(NCORES):
        st = stats_list[c].astype(np.float64)
        for s in range(S):
            b = c * S + s
            sd[b] = st[:, s].sum()
            sa[b, 0] = st[:, 8 + s].sum()
            sa[b, 1] = st[:, 16 + s].sum() + st[0:127, 24 + s].sum()
            sa[b, 2] = st[0:127, 32 + s].sum()
            sa[b, 3] = st[0:63, 40 + s].sum()
            sa[b, 4] = st[0:31, 48 + s].sum()

    counts = np.array(
        [N0 * N0, N1 * N1, N2 * N2, N3 * N3, N4 * N4], np.float64
    )
    l1 = sa / counts  # [B, 5]
    last = np.abs(sd) * float(LAYER_NUM + 1)  # [B]

    # faithful 'fuhao' replication (matches reference.py exactly)
    k_layer = (alpha * np.float32(LAYER_NUM + 2)).astype(np.int32)  # [B]
    trig = k_layer <= LAYER_NUM
    triggered_before = np.concatenate(
        [np.zeros(1, bool), np.cumsum(trig)[:-1] > 0]
    )
    i_idx = np.arange(LAYER_NUM + 1)
    sign = np.where(
        triggered_before[:, None] | (i_idx[None, :] >= k_layer[:, None]),
        1.0,
        -1.0,
    )

    loss_tensor = np.concatenate([l1 * sign, last[:, None]], axis=1)
    return np.float32(loss_tensor.mean())
